# revision 1
# baseline (speedup 1.0000x reference)
"""CRF Viterbi decode (NCRF++-style) on 8 Trainium2 NeuronCores.

Full inputs in, full outputs out. Data-parallel over batch: 128 batch rows
-> 16 per core. Each core runs a bit-exact Viterbi forward scan + backtrace:

  forward:  part_t[b, to] = max_f((emit[t,b,to] + trans[f,to]) + part_{t-1}[b,f])
    - ET tile = transT + emit-bias            (ACT / GPSIMD tensor_scalar)
    - M cube  = ET + part-broadcast           (PE identity / selector matmuls -> PSUM)
    - part_t  = max-reduce over f             (DVE tensor_reduce)
    - part_t  -> DRAM history + PE-transpose to [b, f] staging for the next step
  backtrace: re-derives each argmax bit-exactly from part history + emit +
    gathered transition columns (PE one-hot matmul gather, DVE max8/max_index,
    first-occurrence tie semantics matching jnp.argmax).
"""
import numpy as np
import concourse.bacc as bacc
import concourse.mybir as mybir
import concourse.tile as tile
from concourse.bass_utils import run_bass_kernel_spmd

F32 = mybir.dt.float32
U32 = mybir.dt.uint32
AO = mybir.AluOpType
ACTF = mybir.ActivationFunctionType
AX = mybir.AxisListType

B, T, TAG = 128, 512, 256
START = TAG - 2
STOP = TAG - 1
NCORES = 8
NB = B // NCORES     # 16 batch rows per core
NG = 2               # independent batch groups per core
GB = NB // NG        # 8
CHUNK = 64           # emit chunk (timesteps) per DMA


def _build(T=T, et_engines="ag"):
    NCH = T // CHUNK
    nc = bacc.Bacc("TRN2", num_devices=NCORES, name="crf_viterbi")

    featsT_d = nc.dram_tensor("featsT", [TAG, NB, T], F32, kind="ExternalInput")
    feats0_d = nc.dram_tensor("feats0", [NB, TAG], F32, kind="ExternalInput")
    transT_d = nc.dram_tensor("transT", [TAG, TAG], F32, kind="ExternalInput")
    transS_d = nc.dram_tensor("transS", [1, TAG], F32, kind="ExternalInput")

    score_d = nc.dram_tensor("score", [NB, 1], F32, kind="ExternalOutput")
    decode_d = nc.dram_tensor("decode", [NB, T], U32, kind="ExternalOutput")
    ph_d = nc.dram_tensor("ph", [T, NB, TAG], F32, kind="Internal")

    ident_d = nc.inline_tensor(np.eye(128, dtype=np.float32), name="ident128")
    ones_d = nc.inline_tensor(np.ones((1, 128), dtype=np.float32), name="ones128")
    onescol_d = nc.inline_tensor(np.ones((128, 1), dtype=np.float32), name="onescol")
    iota_np = np.arange(128, dtype=np.float32)[:, None] * np.ones((1, GB), np.float32)
    iota0_d = nc.inline_tensor(iota_np, name="iota0")
    iota1_d = nc.inline_tensor(iota_np + 128.0, name="iota1")
    sel_np = np.zeros((GB, GB * 128), dtype=np.float32)
    for bl in range(GB):
        sel_np[bl, bl * 128:(bl + 1) * 128] = 1.0
    sel_d = nc.inline_tensor(sel_np, name="selbl")
    sel127_np = np.zeros((128, GB), dtype=np.float32)
    sel127_np[127, :] = 1.0
    sel127_d = nc.inline_tensor(sel127_np, name="sel127")

    def engine_of(ch):
        return {"a": nc.scalar, "g": nc.gpsimd, "v": nc.vector}[ch]

    with tile.TileContext(nc) as tc:
        with tc.tile_pool(name="const", bufs=1) as cpool:
            ident = cpool.tile([128, 128], F32, tag="ident")
            nc.sync.dma_start(ident[:], ident_d[:])
            ones = cpool.tile([1, 128], F32, tag="ones")
            nc.sync.dma_start(ones[:], ones_d[:])
            onescol = cpool.tile([128, 1], F32, tag="onescol")
            nc.sync.dma_start(onescol[:], onescol_d[:])
            sel = cpool.tile([GB, GB * 128], F32, tag="sel")
            nc.sync.dma_start(sel[:], sel_d[:])
            sel127 = cpool.tile([128, GB], F32, tag="sel127")
            nc.sync.dma_start(sel127[:], sel127_d[:])
            iota = [cpool.tile([128, GB], F32, tag=f"iota{h}", name=f"iota{h}")
                    for h in range(2)]
            nc.sync.dma_start(iota[0][:], iota0_d[:])
            nc.sync.dma_start(iota[1][:], iota1_d[:])
            transT = [cpool.tile([128, TAG], F32, tag=f"transT{h}", name=f"transT{h}")
                      for h in range(2)]
            for h in range(2):
                nc.sync.dma_start(transT[h][:], transT_d[h * 128:(h + 1) * 128, :])
            transS = cpool.tile([1, TAG], F32, tag="transS")
            nc.sync.dma_start(transS[:], transS_d[:])
            feats0 = [cpool.tile([GB, TAG], F32, tag=f"feats0{g}", name=f"feats0{g}")
                      for g in range(NG)]
            for g in range(NG):
                nc.sync.dma_start(feats0[g][:], feats0_d[g * GB:(g + 1) * GB, :])
            decode_sb = [cpool.tile([GB, T], U32, tag=f"dec{g}", name=f"dec{g}")
                         for g in range(NG)]
            ptrf = [cpool.tile([GB, 1], F32, tag=f"pf{g}", name=f"pf{g}")
                    for g in range(NG)]

            # ---------------- forward ----------------
            with tc.tile_pool(name="emitT", bufs=2) as epool, \
                 tc.tile_pool(name="et", bufs=2) as etpool, \
                 tc.tile_pool(name="stag", bufs=4) as spool, \
                 tc.tile_pool(name="acc", bufs=3) as apool, \
                 tc.tile_pool(name="wave", bufs=2, space="PSUM") as wpool, \
                 tc.tile_pool(name="trps", bufs=3, space="PSUM") as tpool:

                staging = [[None] * NG for _ in range(2)]  # [parity][g]
                for g in range(NG):
                    p0 = tpool.tile([GB, TAG], F32, tag="tr")
                    nc.tensor.matmul(p0[:], ones[:, 0:GB], transS[:],
                                     start=True, stop=True)
                    st = spool.tile([GB, TAG], F32, tag=f"stag{g}")
                    nc.vector.tensor_tensor(out=st[:], in0=feats0[g][:], in1=p0[:],
                                            op=AO.add)
                    nc.sync.dma_start(ph_d[0, g * GB:(g + 1) * GB, :], st[:])
                    staging[0][g] = st

                emitT = {}

                def load_chunk(c):
                    tiles = []
                    for h in range(2):
                        e = epool.tile([128, NB, CHUNK], F32, tag=f"emitT{h}")
                        nc.sync.dma_start(
                            e[:],
                            featsT_d[h * 128:(h + 1) * 128, :, c * CHUNK:(c + 1) * CHUNK])
                        tiles.append(e)
                    return tiles

                emitT[0] = load_chunk(0)
                if NCH > 1:
                    emitT[1] = load_chunk(1)

                for t in range(1, T):
                    c, tl = divmod(t, CHUNK)
                    if tl == 8 and c + 1 < NCH and (c + 1) not in emitT:
                        emitT[c + 1] = load_chunk(c + 1)

                    par, prev = t % 2, (t - 1) % 2
                    et_tiles = [[None] * (NB // 2) for _ in range(2)]
                    k = 0
                    for h in range(2):
                        for p in range(NB // 2):
                            et = etpool.tile([128, 512], F32, tag=f"et{h}_{p}")
                            for j in range(2):
                                b = 2 * p + j
                                eng = engine_of(et_engines[k % len(et_engines)])
                                bias = emitT[c][h][:, b, tl:tl + 1]
                                if eng is nc.scalar:
                                    nc.scalar.activation(
                                        et[:, j * 256:(j + 1) * 256], transT[h][:],
                                        ACTF.Identity, bias=bias, scale=1.0)
                                else:
                                    eng.tensor_scalar(
                                        et[:, j * 256:(j + 1) * 256], transT[h][:],
                                        bias, None, AO.add)
                                k += 1
                            et_tiles[h][p] = et

                    for g in range(NG):
                        stg = staging[prev][g]
                        acc = [apool.tile([128, GB], F32, tag=f"acc{g}{h}",
                                          name=f"acc{g}{h}") for h in range(2)]
                        for h in range(2):
                            for w in range(2):
                                wv = wpool.tile([128, 1024], F32, tag="wave")
                                for q in range(2):
                                    p = g * (GB // 2) + w * 2 + q
                                    nc.tensor.matmul(
                                        wv[:, q * 512:(q + 1) * 512], ident[:],
                                        et_tiles[h][p][:], start=True, stop=False,
                                        skip_group_check=True)
                                for i in range(4):
                                    bl = w * 4 + i
                                    nc.tensor.matmul(
                                        wv[:, i * 256:(i + 1) * 256],
                                        sel[:, bl * 128:(bl + 1) * 128], stg[:],
                                        start=False, stop=True,
                                        skip_group_check=True)
                                nc.vector.tensor_reduce(
                                    out=acc[h][:, w * 4:(w + 1) * 4],
                                    in_=wv[:].rearrange("p (b f) -> p b f", f=256),
                                    op=AO.max, axis=AX.X)
                        st = spool.tile([GB, TAG], F32, tag=f"stag{g}")
                        for h in range(2):
                            tr = tpool.tile([GB, 128], F32, tag="tr")
                            nc.tensor.transpose(tr[:], acc[h][:], ident[:])
                            nc.vector.tensor_copy(st[:, h * 128:(h + 1) * 128], tr[:])
                        nc.sync.dma_start(ph_d[t, g * GB:(g + 1) * GB, :], st[:])
                        staging[par][g] = st

                lastpar = (T - 1) % 2
                for g in range(NG):
                    fin = tpool.tile([GB, TAG], F32, tag="tr")
                    nc.tensor.matmul(fin[:], sel127[:], transT[1][:],
                                     start=True, stop=True)
                    cur = spool.tile([GB, TAG], F32, tag=f"cur{g}")
                    nc.vector.tensor_tensor(out=cur[:], in0=staging[lastpar][g][:],
                                            in1=fin[:], op=AO.add)
                    m8 = apool.tile([GB, 8], F32, tag=f"m8{g}")
                    i8 = apool.tile([GB, 8], U32, tag=f"i8{g}")
                    nc.vector.max(out=m8[:], in_=cur[:])
                    nc.vector.max_index(out=i8[:], in_max=m8[:], in_values=cur[:])
                    sc = apool.tile([GB, 1], F32, tag=f"sc{g}")
                    nc.vector.tensor_copy(sc[:], m8[:, 0:1])
                    nc.sync.dma_start(score_d[g * GB:(g + 1) * GB, :], sc[:])
                    nc.vector.tensor_copy(decode_sb[g][:, T - 1:T], i8[:, 0:1])
                    nc.vector.tensor_copy(ptrf[g][:], i8[:, 0:1])

            # ---------------- backtrace ----------------
            with tc.tile_pool(name="bemit", bufs=2) as bepool, \
                 tc.tile_pool(name="bph", bufs=8) as phpool, \
                 tc.tile_pool(name="bsb", bufs=4) as bpool, \
                 tc.tile_pool(name="bps", bufs=1, space="PSUM") as bpsum:

                bemitT = {}

                def bload_chunk(c):
                    tiles = []
                    for h in range(2):
                        e = bepool.tile([128, NB, CHUNK], F32, tag=f"bemitT{h}")
                        nc.sync.dma_start(
                            e[:],
                            featsT_d[h * 128:(h + 1) * 128, :, c * CHUNK:(c + 1) * CHUNK])
                        tiles.append(e)
                    return tiles

                bemitT[NCH - 1] = bload_chunk(NCH - 1)

                ph_tiles = {}

                def ph_load(s, g):
                    p = phpool.tile([GB, TAG], F32, tag=f"ph{g}", name=f"ph{g}")
                    nc.sync.dma_start(p[:], ph_d[s, g * GB:(g + 1) * GB, :])
                    return p

                for s in range(T - 2, max(T - 5, -1), -1):
                    for g in range(NG):
                        ph_tiles[(s, g)] = ph_load(s, g)

                ptr_cur = [ptrf[g] for g in range(NG)]
                for s in range(T - 1, 0, -1):
                    c, tl = divmod(s, CHUNK)
                    if tl == 8 and c > 0 and (c - 1) not in bemitT:
                        bemitT[c - 1] = bload_chunk(c - 1)
                    pre = s - 4
                    if pre >= 0:
                        for g in range(NG):
                            ph_tiles[(pre, g)] = ph_load(pre, g)
                    for g in range(NG):
                        prow = bpsum.tile([1, GB], F32, tag=f"prow{g}")
                        nc.tensor.transpose(prow[:], ptr_cur[g][:],
                                            ident[0:GB, 0:GB])
                        prow_sb = bpool.tile([1, GB], F32, tag=f"prow_sb{g}")
                        nc.vector.tensor_copy(prow_sb[:], prow[:])
                        pb = bpsum.tile([128, GB], F32, tag=f"pb{g}")
                        nc.tensor.matmul(pb[:], ones[:], prow_sb[:],
                                         start=True, stop=True)
                        oh = [bpool.tile([128, GB], F32, tag=f"oh{g}{h}",
                                         name=f"oh{g}{h}") for h in range(2)]
                        em = [bpool.tile([128, GB], F32, tag=f"em{g}{h}",
                                         name=f"em{g}{h}") for h in range(2)]
                        for h in range(2):
                            nc.vector.tensor_tensor(out=oh[h][:], in0=iota[h][:],
                                                    in1=pb[:], op=AO.is_equal)
                            nc.vector.tensor_tensor(
                                out=em[h][:], in0=oh[h][:],
                                in1=bemitT[c][h][:, g * GB:(g + 1) * GB, tl],
                                op=AO.mult)
                        gcol = bpsum.tile([GB, TAG], F32, tag=f"gcol{g}")
                        ecol = bpsum.tile([GB, 1], F32, tag=f"ecol{g}")
                        for h in range(2):
                            nc.tensor.matmul(gcol[:], oh[h][:], transT[h][:],
                                             start=(h == 0), stop=(h == 1))
                            nc.tensor.matmul(ecol[:], em[h][:], onescol[:],
                                             start=(h == 0), stop=(h == 1))
                        etg = bpool.tile([GB, TAG], F32, tag=f"etg{g}")
                        nc.vector.tensor_scalar(etg[:], gcol[:], ecol[:], None, AO.add)
                        cur = bpool.tile([GB, TAG], F32, tag=f"bcur{g}")
                        nc.vector.tensor_tensor(out=cur[:], in0=etg[:],
                                                in1=ph_tiles.pop((s - 1, g))[:],
                                                op=AO.add)
                        m8 = bpool.tile([GB, 8], F32, tag=f"bm8{g}")
                        i8 = bpool.tile([GB, 8], U32, tag=f"bi8{g}")
                        nc.vector.max(out=m8[:], in_=cur[:])
                        nc.vector.max_index(out=i8[:], in_max=m8[:], in_values=cur[:])
                        nc.vector.tensor_copy(decode_sb[g][:, s - 1:s], i8[:, 0:1])
                        pf = bpool.tile([GB, 1], F32, tag=f"bpf{g}")
                        nc.vector.tensor_copy(pf[:], i8[:, 0:1])
                        ptr_cur[g] = pf

                for g in range(NG):
                    nc.sync.dma_start(decode_d[g * GB:(g + 1) * GB, :],
                                      decode_sb[g][:])

    nc.compile()
    return nc


_NC_CACHE = {}


def _get_nc(**kw):
    key = tuple(sorted(kw.items()))
    if key not in _NC_CACHE:
        _NC_CACHE[key] = _build(**kw)
    return _NC_CACHE[key]


def _viterbi_numpy(in_features, transitions, mask):
    """General-mask fallback; mirrors reference.py exactly."""
    Bn, Tn, TAGn = in_features.shape
    STARTn, STOPn = TAGn - 2, TAGn - 1
    lengths = mask.sum(axis=1).astype(np.int32)
    featsT = np.transpose(in_features, (1, 0, 2))
    maskT = np.transpose(mask, (1, 0))
    part = (featsT[0] + transitions[STARTn][None, :]).astype(np.float32)
    part_hist = [part]
    bps = []
    for t in range(1, Tn):
        cur = (featsT[t][:, None, :] + transitions[None]).astype(np.float32)
        cur = (cur + part[:, :, None]).astype(np.float32)
        part = cur.max(axis=1)
        bp = cur.argmax(axis=1).astype(np.int32)
        bps.append(np.where(maskT[t][:, None] > 0, bp, 0))
        part_hist.append(part)
    part_hist = np.stack(part_hist)
    last_idx = lengths - 1
    last_partition = part_hist[last_idx, np.arange(Bn)]
    last_values = (last_partition[:, :, None] + transitions[None]).astype(np.float32)
    path_vals = last_values.max(axis=1)
    last_bp = last_values.argmax(axis=1).astype(np.int32)
    path_score = path_vals[:, STOPn][:, None].astype(np.float32)
    pointer = last_bp[:, STOPn]
    bp_seq = np.concatenate([np.stack(bps), np.zeros((1, Bn, TAGn), np.int32)], 0)
    bp_seq = np.transpose(bp_seq, (1, 0, 2)).copy()
    bp_seq[np.arange(Bn), last_idx, :] = pointer[:, None]
    bp_seq = np.transpose(bp_seq, (1, 0, 2))
    ptr = pointer.copy()
    ptrs = []
    for t in range(Tn - 2, -1, -1):
        ptr = bp_seq[t][np.arange(Bn), ptr]
        ptrs.append(ptr)
    ptrs.reverse()
    decode_idx = np.stack(ptrs + [pointer], axis=0)
    return path_score, np.transpose(decode_idx, (1, 0)).astype(np.int32)


def kernel(in_features, mask, transitions, _trace=False, _trace_kwargs=None):
    in_features = np.asarray(in_features, dtype=np.float32)
    mask = np.asarray(mask)
    transitions = np.asarray(transitions, dtype=np.float32)
    assert in_features.shape == (B, T, TAG), in_features.shape

    if not np.all(mask == 1):
        return _viterbi_numpy(in_features, transitions, mask)

    nc = _get_nc()
    transT = np.ascontiguousarray(transitions.T)
    transS = np.ascontiguousarray(transitions[START:START + 1, :])
    in_maps = []
    for cc in range(NCORES):
        fc = in_features[cc * NB:(cc + 1) * NB]          # [NB, T, TAG]
        in_maps.append({
            "featsT": np.ascontiguousarray(fc.transpose(2, 0, 1)),
            "feats0": np.ascontiguousarray(fc[:, 0, :]),
            "transT": transT,
            "transS": transS,
        })
    kw = {}
    if _trace:
        kw["trace"] = True
        if _trace_kwargs:
            kw.update(_trace_kwargs)
    res = run_bass_kernel_spmd(nc, in_maps, core_ids=list(range(NCORES)), **kw)
    score = np.concatenate([res.results[cc]["score"] for cc in range(NCORES)], 0)
    decode = np.concatenate([res.results[cc]["decode"] for cc in range(NCORES)], 0)
    out = (score.astype(np.float32), decode.astype(np.int32))
    if _trace:
        return out, res
    return out


# revision 2
# speedup vs baseline: 3.5067x; 3.5067x over previous
"""CRF Viterbi decode (NCRF++-style) on 8 Trainium2 NeuronCores.

Full inputs in, full outputs out. Data-parallel over batch: 128 rows -> 16 per
core, two independent 8-row recurrence chains per core. Bit-exact vs the jax
reference (identical f32 op association, first-occurrence argmax ties).

Forward, per timestep (per core):
  part_t[b, to] = max_f((emit[t,b,to] + trans[f,to]) + part_{t-1}[b,f])
  - PE broadcasts part rows into PSUM via selector-weight matmuls, 4 packed
    concurrently in 32-row tile positions.
  - One fused custom DVE op per (batch, tag-half) computes
    (transT*1 + emit_col) + part_bcast with a fused max fold -> part_t column.
  - PE transposes the accumulated [to, b] block back to [b, to] staging; DMA
    replicates staging to the four 32-row offsets and streams it to DRAM
    history for the backtrace.
Backtrace re-derives each argmax bit-exactly: PE one-hot matmuls gather the
transition column and emission scalar, the fused DVE op rebuilds the candidate
row, and max8/max_index (first-index tie semantics) step the pointer back.
"""
import numpy as np
import concourse.bacc as bacc
import concourse.mybir as mybir
import concourse.tile as tile
from concourse.bass_utils import run_bass_kernel_spmd

F32 = mybir.dt.float32
U32 = mybir.dt.uint32
AO = mybir.AluOpType
ACTF = mybir.ActivationFunctionType
AX = mybir.AxisListType

B, T, TAG = 128, 512, 256
START = TAG - 2
STOP = TAG - 1
NCORES = 8
NB = B // NCORES     # 16 batch rows per core
NG = 2               # independent batch groups per core
GB = NB // NG        # 8
CHUNK = 64           # emit chunk (timesteps) per DMA

_OPNAME = "CRF_AFFINE_ADD_MAX"


def _register_dve_op():
    """out = (in0*s0 + s1) + in1 ; accum_out = max_k out  (init -FLT_MAX)."""
    from concourse import dve_ops
    from concourse.dve_spec import Spec, Src0, Src1, C0, C1, AluOp, lower
    from concourse.dve_table_gen import DveOpSpec

    for op in dve_ops.OPS:
        if op.name == _OPNAME:
            return op

    def _ref(in0, in1, s0, s1, imm2):
        body = ((in0.astype(np.float32) * s0 + s1) + in1).astype(np.float32)
        acc = body.reshape(body.shape[0], -1).max(axis=-1, keepdims=True)
        return body, np.maximum(acc, np.float32(-np.finfo(np.float32).max))

    spec = Spec(body=(Src0 * C0 + C1) + Src1, accum=AluOp.MAX, reference=_ref)
    row = dve_ops._CUSTOM_DVE_ROW_BASE + len(dve_ops.OPS)
    assert row < 0x20
    dve_ops._SUB_OPCODE_FOR_NAME[_OPNAME] = row
    shas = {}
    for ver in ("v3", "v4"):
        try:
            uops = lower(spec, ver=ver)
        except Exception:
            continue
        s = DveOpSpec(name=_OPNAME, opcode=row, uops=uops, rd1_en=True)
        shas[ver] = s.sha(ver)
    op = dve_ops.DveOp(_OPNAME, spec, subdim=False, uops_sha=shas)
    dve_ops.OPS.append(op)
    dve_ops.CUSTOM_DVE_SPECS[_OPNAME] = spec
    return op


def _build(T=T):
    OP = _register_dve_op()
    NCH = T // CHUNK
    nc = bacc.Bacc("TRN2", num_devices=NCORES, name="crf_viterbi")

    featsT_d = nc.dram_tensor("featsT", [TAG, NB, T], F32, kind="ExternalInput")
    feats0_d = nc.dram_tensor("feats0", [NB, TAG], F32, kind="ExternalInput")
    transT_d = nc.dram_tensor("transT", [TAG, TAG], F32, kind="ExternalInput")
    transS_d = nc.dram_tensor("transS", [1, TAG], F32, kind="ExternalInput")

    score_d = nc.dram_tensor("score", [NB, 1], F32, kind="ExternalOutput")
    decode_d = nc.dram_tensor("decode", [NB, T], U32, kind="ExternalOutput")
    ph_d = nc.dram_tensor("ph", [T, NB, TAG], F32, kind="Internal")

    ident_d = nc.inline_tensor(np.eye(128, dtype=np.float32), name="ident128")
    ones_d = nc.inline_tensor(np.ones((1, 128), dtype=np.float32), name="ones128")
    onescol_d = nc.inline_tensor(np.ones((128, 1), dtype=np.float32), name="onescol")
    iota_np = np.arange(128, dtype=np.float32)[:, None] * np.ones((1, GB), np.float32)
    iota0_d = nc.inline_tensor(iota_np, name="iota0")
    iota1_d = nc.inline_tensor(iota_np + 128.0, name="iota1")
    # selector weights at 4 row-group offsets: sel4[32g+k, bl*128+m] = (k == bl)
    sel4_np = np.zeros((128, GB * 128), dtype=np.float32)
    for g in range(4):
        for bl in range(GB):
            sel4_np[g * 32 + bl, bl * 128:(bl + 1) * 128] = 1.0
    sel4_d = nc.inline_tensor(sel4_np, name="sel4")
    sel127_np = np.zeros((128, GB), dtype=np.float32)
    sel127_np[127, :] = 1.0
    sel127_d = nc.inline_tensor(sel127_np, name="sel127")

    with tile.TileContext(nc) as tc:
        with tc.tile_pool(name="const", bufs=1) as cpool:
            ident = cpool.tile([128, 128], F32, tag="ident")
            nc.sync.dma_start(ident[:], ident_d[:])
            ones = cpool.tile([1, 128], F32, tag="ones")
            nc.sync.dma_start(ones[:], ones_d[:])
            onescol = cpool.tile([128, 1], F32, tag="onescol")
            nc.sync.dma_start(onescol[:], onescol_d[:])
            sel4 = cpool.tile([128, GB * 128], F32, tag="sel4")
            nc.sync.dma_start(sel4[:], sel4_d[:])
            sel127 = cpool.tile([128, GB], F32, tag="sel127")
            nc.sync.dma_start(sel127[:], sel127_d[:])
            iota = [cpool.tile([128, GB], F32, tag=f"iota{h}", name=f"iota{h}")
                    for h in range(2)]
            nc.sync.dma_start(iota[0][:], iota0_d[:])
            nc.sync.dma_start(iota[1][:], iota1_d[:])
            transT = [cpool.tile([128, TAG], F32, tag=f"transT{h}", name=f"transT{h}")
                      for h in range(2)]
            for h in range(2):
                nc.sync.dma_start(transT[h][:], transT_d[h * 128:(h + 1) * 128, :])
            transS = cpool.tile([1, TAG], F32, tag="transS")
            nc.sync.dma_start(transS[:], transS_d[:])
            feats0 = [cpool.tile([GB, TAG], F32, tag=f"feats0{g}", name=f"feats0{g}")
                      for g in range(NG)]
            for g in range(NG):
                nc.sync.dma_start(feats0[g][:], feats0_d[g * GB:(g + 1) * GB, :])
            decode_sb = [cpool.tile([GB, T], U32, tag=f"dec{g}", name=f"dec{g}")
                         for g in range(NG)]
            ptrf = [cpool.tile([GB, 1], F32, tag=f"pf{g}", name=f"pf{g}")
                    for g in range(NG)]

            # ---------------- forward ----------------
            with tc.tile_pool(name="emitT", bufs=2) as epool, \
                 tc.tile_pool(name="stag", bufs=3) as spool, \
                 tc.tile_pool(name="acc", bufs=3) as apool, \
                 tc.tile_pool(name="mout", bufs=4) as mpool, \
                 tc.tile_pool(name="bc", bufs=5, space="PSUM") as bcpool, \
                 tc.tile_pool(name="trps", bufs=2, space="PSUM") as tpool:

                # t=0: part0 = feats0 + trans[START] bcast; replicate to 4 offsets
                staging = [[None] * NG for _ in range(2)]  # [parity][g]
                for g in range(NG):
                    p0 = bcpool.tile([GB, TAG], F32, tag="bc")
                    nc.tensor.matmul(p0[:], ones[:, 0:GB], transS[:],
                                     start=True, stop=True)
                    s4 = spool.tile([128, TAG], F32, tag=f"stag{g}",
                                    name=f"stag{g}")
                    nc.vector.tensor_tensor(out=s4[0:GB, :], in0=feats0[g][:],
                                            in1=p0[:], op=AO.add)
                    for r in range(1, 4):
                        nc.sync.dma_start(s4[32 * r:32 * r + GB, :], s4[0:GB, :])
                    nc.sync.dma_start(ph_d[0, g * GB:(g + 1) * GB, :], s4[0:GB, :])
                    staging[0][g] = s4

                emitT = {}

                def load_chunk(c):
                    tiles = []
                    for h in range(2):
                        e = epool.tile([128, NB, CHUNK], F32, tag=f"emitT{h}",
                                       name=f"emitT{h}")
                        nc.sync.dma_start(
                            e[:],
                            featsT_d[h * 128:(h + 1) * 128, :, c * CHUNK:(c + 1) * CHUNK])
                        tiles.append(e)
                    return tiles

                emitT[0] = load_chunk(0)
                if NCH > 1:
                    emitT[1] = load_chunk(1)

                for t in range(1, T):
                    c, tl = divmod(t, CHUNK)
                    if tl == 8 and c + 1 < NCH and (c + 1) not in emitT:
                        emitT[c + 1] = load_chunk(c + 1)

                    par, prev = t % 2, (t - 1) % 2
                    for g in range(NG):
                        s4p = staging[prev][g]
                        acc = [apool.tile([128, GB], F32, tag=f"acc{g}{h}",
                                          name=f"acc{g}{h}") for h in range(2)]
                        for bl in range(GB):
                            rg = bl % 4  # row-group for tile-position packing
                            bc = bcpool.tile([128, TAG], F32, tag="bc")
                            nc.tensor.matmul(
                                bc[:], sel4[32 * rg:32 * rg + GB,
                                            bl * 128:(bl + 1) * 128],
                                s4p[32 * rg:32 * rg + GB, :],
                                start=True, stop=True, tile_position=(32 * rg, 0),
                                skip_group_check=True)
                            b = g * GB + bl
                            for h in range(2):
                                mo = mpool.tile([128, TAG], F32, tag=f"mo{h}",
                                                name=f"mo{h}")
                                nc.vector._custom_dve(
                                    OP, out=mo[:], in0=transT[h][:], in1=bc[:],
                                    s0=1.0, s1=emitT[c][h][:, b, tl:tl + 1],
                                    accum_out=acc[h][:, bl:bl + 1])
                        s4 = spool.tile([128, TAG], F32, tag=f"stag{g}",
                                        name=f"stag{g}")
                        for h in range(2):
                            tr = tpool.tile([GB, 128], F32, tag="tr")
                            nc.tensor.transpose(tr[:], acc[h][:], ident[:])
                            nc.scalar.copy(s4[0:GB, h * 128:(h + 1) * 128], tr[:])
                        for r in range(1, 4):
                            nc.sync.dma_start(s4[32 * r:32 * r + GB, :], s4[0:GB, :])
                        nc.sync.dma_start(ph_d[t, g * GB:(g + 1) * GB, :], s4[0:GB, :])
                        staging[par][g] = s4

                lastpar = (T - 1) % 2
                for g in range(NG):
                    fin = bcpool.tile([GB, TAG], F32, tag="bc")
                    nc.tensor.matmul(fin[:], sel127[:], transT[1][:],
                                     start=True, stop=True)
                    cur = spool.tile([GB, TAG], F32, tag=f"cur{g}", name=f"cur{g}")
                    nc.vector.tensor_tensor(out=cur[:],
                                            in0=staging[lastpar][g][0:GB, :],
                                            in1=fin[:], op=AO.add)
                    m8 = apool.tile([GB, 8], F32, tag=f"m8{g}", name=f"m8{g}")
                    i8 = apool.tile([GB, 8], U32, tag=f"i8{g}", name=f"i8{g}")
                    nc.vector.max(out=m8[:], in_=cur[:])
                    nc.vector.max_index(out=i8[:], in_max=m8[:], in_values=cur[:])
                    sc = apool.tile([GB, 1], F32, tag=f"sc{g}", name=f"sc{g}")
                    nc.vector.tensor_copy(sc[:], m8[:, 0:1])
                    nc.sync.dma_start(score_d[g * GB:(g + 1) * GB, :], sc[:])
                    nc.vector.tensor_copy(decode_sb[g][:, T - 1:T], i8[:, 0:1])
                    nc.vector.tensor_copy(ptrf[g][:], i8[:, 0:1])

            # ---------------- backtrace ----------------
            with tc.tile_pool(name="bemit", bufs=2) as bepool, \
                 tc.tile_pool(name="bph", bufs=8) as phpool, \
                 tc.tile_pool(name="bsb", bufs=4) as bpool, \
                 tc.tile_pool(name="bps", bufs=1, space="PSUM") as bpsum:

                bemitT = {}

                def bload_chunk(c):
                    tiles = []
                    for h in range(2):
                        e = bepool.tile([128, NB, CHUNK], F32, tag=f"bemitT{h}",
                                        name=f"bemitT{h}")
                        nc.sync.dma_start(
                            e[:],
                            featsT_d[h * 128:(h + 1) * 128, :, c * CHUNK:(c + 1) * CHUNK])
                        tiles.append(e)
                    return tiles

                bemitT[NCH - 1] = bload_chunk(NCH - 1)

                ph_tiles = {}

                def ph_load(s, g):
                    p = phpool.tile([GB, TAG], F32, tag=f"ph{g}", name=f"ph{g}")
                    nc.sync.dma_start(p[:], ph_d[s, g * GB:(g + 1) * GB, :])
                    return p

                for s in range(T - 2, max(T - 5, -1), -1):
                    for g in range(NG):
                        ph_tiles[(s, g)] = ph_load(s, g)

                ptr_cur = [ptrf[g] for g in range(NG)]
                dummy_acc = cpool.tile([GB, 1], F32, tag="dacc")
                for s in range(T - 1, 0, -1):
                    c, tl = divmod(s, CHUNK)
                    if tl == 8 and c > 0 and (c - 1) not in bemitT:
                        bemitT[c - 1] = bload_chunk(c - 1)
                    pre = s - 4
                    if pre >= 0:
                        for g in range(NG):
                            ph_tiles[(pre, g)] = ph_load(pre, g)
                    for g in range(NG):
                        prow = bpsum.tile([1, GB], F32, tag=f"prow{g}",
                                          name=f"prow{g}")
                        nc.tensor.transpose(prow[:], ptr_cur[g][:],
                                            ident[0:GB, 0:GB])
                        prow_sb = bpool.tile([1, GB], F32, tag=f"prow_sb{g}",
                                             name=f"prow_sb{g}")
                        nc.scalar.copy(prow_sb[:], prow[:])
                        pb = bpsum.tile([128, GB], F32, tag=f"pb{g}", name=f"pb{g}")
                        nc.tensor.matmul(pb[:], ones[:], prow_sb[:],
                                         start=True, stop=True)
                        oh = [bpool.tile([128, GB], F32, tag=f"oh{g}{h}",
                                         name=f"oh{g}{h}") for h in range(2)]
                        em = [bpool.tile([128, GB], F32, tag=f"em{g}{h}",
                                         name=f"em{g}{h}") for h in range(2)]
                        for h in range(2):
                            nc.vector.tensor_tensor(out=oh[h][:], in0=iota[h][:],
                                                    in1=pb[:], op=AO.is_equal)
                            nc.vector.tensor_tensor(
                                out=em[h][:], in0=oh[h][:],
                                in1=bemitT[c][h][:, g * GB:(g + 1) * GB, tl],
                                op=AO.mult)
                        gcol = bpsum.tile([GB, TAG], F32, tag=f"gcol{g}",
                                          name=f"gcol{g}")
                        ecol = bpsum.tile([GB, 1], F32, tag=f"ecol{g}",
                                          name=f"ecol{g}")
                        for h in range(2):
                            nc.tensor.matmul(gcol[:], oh[h][:], transT[h][:],
                                             start=(h == 0), stop=(h == 1))
                            nc.tensor.matmul(ecol[:], em[h][:], onescol[:],
                                             start=(h == 0), stop=(h == 1))
                        cur = bpool.tile([GB, TAG], F32, tag=f"bcur{g}",
                                         name=f"bcur{g}")
                        nc.vector._custom_dve(
                            OP, out=cur[:], in0=gcol[:],
                            in1=ph_tiles.pop((s - 1, g))[:],
                            s0=1.0, s1=ecol[:], accum_out=dummy_acc[:])
                        m8 = bpool.tile([GB, 8], F32, tag=f"bm8{g}", name=f"bm8{g}")
                        i8 = bpool.tile([GB, 8], U32, tag=f"bi8{g}", name=f"bi8{g}")
                        nc.vector.max(out=m8[:], in_=cur[:])
                        nc.vector.max_index(out=i8[:], in_max=m8[:], in_values=cur[:])
                        nc.vector.tensor_copy(decode_sb[g][:, s - 1:s], i8[:, 0:1])
                        pf = bpool.tile([GB, 1], F32, tag=f"bpf{g}", name=f"bpf{g}")
                        nc.vector.tensor_copy(pf[:], i8[:, 0:1])
                        ptr_cur[g] = pf

                for g in range(NG):
                    nc.sync.dma_start(decode_d[g * GB:(g + 1) * GB, :],
                                      decode_sb[g][:])

    nc.compile()
    return nc


_NC_CACHE = {}


def _get_nc(**kw):
    key = tuple(sorted(kw.items()))
    if key not in _NC_CACHE:
        _NC_CACHE[key] = _build(**kw)
    return _NC_CACHE[key]


def _viterbi_numpy(in_features, transitions, mask):
    """General-mask fallback; mirrors reference.py exactly."""
    Bn, Tn, TAGn = in_features.shape
    STARTn, STOPn = TAGn - 2, TAGn - 1
    lengths = mask.sum(axis=1).astype(np.int32)
    featsT = np.transpose(in_features, (1, 0, 2))
    maskT = np.transpose(mask, (1, 0))
    part = (featsT[0] + transitions[STARTn][None, :]).astype(np.float32)
    part_hist = [part]
    bps = []
    for t in range(1, Tn):
        cur = (featsT[t][:, None, :] + transitions[None]).astype(np.float32)
        cur = (cur + part[:, :, None]).astype(np.float32)
        part = cur.max(axis=1)
        bp = cur.argmax(axis=1).astype(np.int32)
        bps.append(np.where(maskT[t][:, None] > 0, bp, 0))
        part_hist.append(part)
    part_hist = np.stack(part_hist)
    last_idx = lengths - 1
    last_partition = part_hist[last_idx, np.arange(Bn)]
    last_values = (last_partition[:, :, None] + transitions[None]).astype(np.float32)
    path_vals = last_values.max(axis=1)
    last_bp = last_values.argmax(axis=1).astype(np.int32)
    path_score = path_vals[:, STOPn][:, None].astype(np.float32)
    pointer = last_bp[:, STOPn]
    bp_seq = np.concatenate([np.stack(bps), np.zeros((1, Bn, TAGn), np.int32)], 0)
    bp_seq = np.transpose(bp_seq, (1, 0, 2)).copy()
    bp_seq[np.arange(Bn), last_idx, :] = pointer[:, None]
    bp_seq = np.transpose(bp_seq, (1, 0, 2))
    ptr = pointer.copy()
    ptrs = []
    for t in range(Tn - 2, -1, -1):
        ptr = bp_seq[t][np.arange(Bn), ptr]
        ptrs.append(ptr)
    ptrs.reverse()
    decode_idx = np.stack(ptrs + [pointer], axis=0)
    return path_score, np.transpose(decode_idx, (1, 0)).astype(np.int32)


def kernel(in_features, mask, transitions, _trace=False, _trace_kwargs=None):
    in_features = np.asarray(in_features, dtype=np.float32)
    mask = np.asarray(mask)
    transitions = np.asarray(transitions, dtype=np.float32)
    assert in_features.shape == (B, T, TAG), in_features.shape

    if not np.all(mask == 1):
        return _viterbi_numpy(in_features, transitions, mask)

    nc = _get_nc()
    transT = np.ascontiguousarray(transitions.T)
    transS = np.ascontiguousarray(transitions[START:START + 1, :])
    in_maps = []
    for cc in range(NCORES):
        fc = in_features[cc * NB:(cc + 1) * NB]          # [NB, T, TAG]
        in_maps.append({
            "featsT": np.ascontiguousarray(fc.transpose(2, 0, 1)),
            "feats0": np.ascontiguousarray(fc[:, 0, :]),
            "transT": transT,
            "transS": transS,
        })
    kw = {}
    if _trace:
        kw["trace"] = True
        if _trace_kwargs:
            kw.update(_trace_kwargs)
    res = run_bass_kernel_spmd(nc, in_maps, core_ids=list(range(NCORES)), **kw)
    score = np.concatenate([res.results[cc]["score"] for cc in range(NCORES)], 0)
    decode = np.concatenate([res.results[cc]["decode"] for cc in range(NCORES)], 0)
    out = (score.astype(np.float32), decode.astype(np.int32))
    if _trace:
        return out, res
    return out
